# revision 48
# baseline (speedup 1.0000x reference)
"""Segment-mean + projection kernel for Trainium2 (8 NeuronCores, SPMD).

logits[b] = (mean of x rows in bag b) @ rel_weight.T + bias

Strategy: data-parallel over bags, two precision streams per core.

Large bags (count >= SMALL_T) go through an fp8-e4m3 path: rows are packed
into 768-row groups (3 pairs of 128-row tiles), the per-tile one-hot
matrices are precomputed on the host (exact 0/1 values in fp8) and DMA'd
alongside x, and the PE accumulates one-hot.T @ x with DoubleRow fp8
matmuls (two 128-row tiles contracted per pass at 0.5 cycles/column).
Small bags go through a classic fp16 path (256-row groups) because fp8
quantization error scales as 1/sqrt(count) and breaks the accuracy gate
for tiny bags.

Groups hold only whole bags (no bag straddles a group boundary), so there
is no fixup pass and no cross-group dependency. Per group the three
512-col PSUM bank regions are scaled by 1/count and copied to SBUF in one
activation, transposed chunk-wise by the PE into a single PSUM tile, and
copied once (768 cols) to the mgt staging buffer. Every two groups the
relation matrix is applied (6 accumulating matmuls) and bias added.
Host assembles the final [n_bags, 53] output from per-slot columns.
"""
import sys
import re

sys.path.insert(0, "/opt/trn_rl_repo")

import numpy as np
import ml_dtypes

N_CORES = 8
USE_DR = True  # DoubleRow fp8 matmuls for the large-bag stream
SMALL_T = 5  # bags with count < SMALL_T take the fp16 path
ROWS8 = 768  # rows per fp8 group: 3 DoubleRow pairs of 256
ROWS16 = 256  # rows per fp16 group: 2 tiles of 128
MAX_BAGS = 128  # output slots per group (PSUM partitions)
D = 690
SPLIT = 230  # 3 PSUM splits of 230 cols (each within one 2KB bank)
C = 53
D_CHUNKS = 6  # ceil(690 / 128); last chunk is 50 wide
D_LAST = D - 5 * 128  # 50

F8 = ml_dtypes.float8_e4m3


def _apply_walrus_workarounds():
    """This walrus build allows at most one semaphore wait per instruction
    on several opcodes (Drain, Matmult/LDW). Patch Tile's tail drain to use
    standalone wait_ge instructions, and provide a post-pass that hoists
    excess waits onto InstNoOp instructions."""
    from concourse import tile, mybir

    def _patched_drain_and_barrier(self, tick_clock, wait_clock):
        gc = tick_clock.global_clock
        ticks = [int(s) for s in re.findall(r"\d+", repr(gc))]
        allocated = self.sems.allocated()
        for proc, sem in sorted(allocated.items()):
            t = ticks[proc] if proc < len(ticks) else 0
            if t > 0:
                mult = 16 if "DMA" in sem.name else 1
                self.nc.sync.wait_ge(sem, t * mult)
        self.nc.sync.drain()
        self.nc.all_engine_barrier()
        popped = self.nc._tile_sem_poison_stack.pop()
        assert popped is self._sem_poison
        # One-shot outermost context: skip the semaphore RANGE_CLEAR and
        # the second barrier (only needed for NEFF re-execution / nested
        # contexts); just do the python-side bookkeeping.
        sem_nums = [s.num for s in allocated.values()]
        self.nc._state.prepend_free_semaphores(sem_nums)
        for poison_set in self.nc._tile_sem_poison_stack:
            poison_set.update(sem_nums)

    tile.TileContext._drain_and_barrier = _patched_drain_and_barrier

    def split_multi_waits(nc, max_waits=1):
        for f in nc.m.functions:
            for b in f.blocks:
                insts = list(b.instructions)
                new = []
                dirty = False
                for inst in insts:
                    si = inst.sync_info
                    if si is not None and len(si.on_wait) > max_waits:
                        waits = list(si.on_wait)
                        extra, keep = waits[:-max_waits], waits[-max_waits:]
                        for k, w in enumerate(extra):
                            nop = mybir.InstNoOp(
                                name=f"{inst.name}-hw{k}", ins=[], outs=[]
                            )
                            nop.engine = inst.engine
                            nop.sync_info = mybir.SyncInfo(
                                on_wait=[w], on_update=[]
                            )
                            new.append(nop)
                        inst.sync_info = mybir.SyncInfo(
                            on_wait=keep, on_update=list(si.on_update)
                        )
                        dirty = True
                    new.append(inst)
                if dirty:
                    b.instructions = new

    def dedupe_ldweights(nc):
        """Drop back-to-back InstLdweights that reload identical weights
        (the 2nd/3rd PSUM-split matmuls of a pair share the one-hot lhsT).
        The PE array retains weights across matmuls, LDWs neither wait nor
        tick tile semaphores (only the first of a run carries waits), so
        removal is safe and saves ~0.2us of PE issue per pair."""
        removed = 0
        for f in nc.m.functions:
            for b in f.blocks:
                out = []
                last_key = None
                for inst in b.instructions:
                    if isinstance(inst, mybir.InstLdweights):
                        ap = inst.ins[0]
                        key = (
                            str(getattr(ap, "memref", "")),
                            getattr(ap, "offset", None),
                            str(getattr(ap, "ap", "")),
                            str(getattr(ap, "dtype", "")),
                            str(getattr(inst, "perf_mode", None)),
                            str(getattr(inst, "tile_position", None)),
                        )
                        si = inst.sync_info
                        clean = si is None or (
                            not si.on_wait and not si.on_update
                        )
                        if key == last_key and clean:
                            removed += 1
                            continue
                        last_key = key
                    out.append(inst)
                b.instructions = out
        return removed

    return split_multi_waits, dedupe_ldweights


def _pack(cnts, max_rows, max_bags):
    """Greedy: consecutive bags into groups of <= max_rows rows and
    < max_bags bags. Returns per-bag group id and group count."""
    n = len(cnts)
    if n == 0:
        return np.zeros(0, np.int64), 0
    gids = np.zeros(n, np.int64)
    g = 0
    rows = 0
    nb = 0
    for i in range(n):
        c = int(cnts[i])
        if rows + c > max_rows or nb >= max_bags:
            g += 1
            rows = 0
            nb = 0
        gids[i] = g
        rows += c
        nb += 1
    return gids, g + 1


def _rows_of(scope, counts, stream_bags, gids, ng):
    """Per-row (group, rank-in-group, slot, global-row-id) for a stream."""
    nb = len(stream_bags)
    first = np.searchsorted(gids, np.arange(ng))
    slot_of_bag = np.arange(nb) - first[gids]
    cnts = counts[stream_bags]
    rows_per_g = np.bincount(gids, weights=cnts, minlength=ng).astype(np.int64)
    row_start_g = np.concatenate([[0], np.cumsum(rows_per_g)])[:-1]
    row_bag_rank = np.repeat(np.arange(nb), cnts)
    g_of_row = gids[row_bag_rank]
    n_rows = int(cnts.sum())
    row_rank = np.arange(n_rows) - row_start_g[g_of_row]
    bag_row0 = scope[stream_bags]
    within = np.arange(n_rows) - np.repeat(
        np.concatenate([[0], np.cumsum(cnts)])[:-1], cnts
    )
    grows = (bag_row0[row_bag_rank] + within).astype(np.int64)
    return g_of_row, row_rank, slot_of_bag[row_bag_rank], grows, slot_of_bag


def _stream_arrays(x, scope, stream_bags, counts, n_groups, hilo):
    """Build padded fp8 x + one-hot arrays for one stream of one core.

    Full stream (hilo=False): 768 rows/group, row slot r -> pair
    q = r // 256, tile j = (r // 128) % 2, partition p = r % 128.
    Hi/lo stream (hilo=True): 384 rows/group at double width, row
    r -> pair q = r // 128, partition p = r % 128; tile A holds
    e4m3(x), tile B holds e4m3(x - hi), the one-hot is duplicated in
    both halves so DoubleRow accumulates hi + lo = x at ~11-bit
    mantissa precision (for the small bags fp8 alone is too lossy).
    Returns (xoh [n_groups*128, 4908], recip [n_groups, 128],
    slot2bag [n_groups, 128]).
    """
    nb = len(stream_bags)
    rows_cap = ROWS8 // 2 if hilo else ROWS8
    gids, ng = _pack(counts[stream_bags], rows_cap, MAX_BAGS)
    assert ng <= n_groups
    X = np.zeros((n_groups, 3, 2, 128, D), dtype=F8)
    OH = np.zeros((n_groups, 3, 2, 128, 128), dtype=F8)
    recip = np.ones((n_groups, 128), dtype=np.float32)
    slot2bag = np.full((n_groups, 128), -1, dtype=np.int64)
    if nb:
        g_of_row, row_rank, slot_of_row, grows, slot_of_bag = _rows_of(
            scope, counts, stream_bags, gids, ng
        )
        xs = x[grows]
        if hilo:
            hi = xs.astype(F8)
            lo = (xs - hi.astype(np.float32)).astype(F8)
            q, p = row_rank // 128, row_rank % 128
            X[g_of_row, q, 0, p] = hi
            X[g_of_row, q, 1, p] = lo
            OH[g_of_row, q, 0, p, slot_of_row] = 1.0
            OH[g_of_row, q, 1, p, slot_of_row] = 1.0
        else:
            q, j, p = row_rank // 256, (row_rank // 128) % 2, row_rank % 128
            X[g_of_row, q, j, p] = xs.astype(F8)
            OH[g_of_row, q, j, p, slot_of_row] = 1.0
        cnts = counts[stream_bags]
        recip[gids, slot_of_bag] = (1.0 / cnts).astype(np.float32)
        slot2bag[gids, slot_of_bag] = stream_bags
    Xr = np.ascontiguousarray(X.transpose(0, 3, 1, 2, 4)).reshape(
        n_groups * 128, 6 * D
    )
    OHr = np.ascontiguousarray(OH.transpose(0, 3, 1, 2, 4)).reshape(
        n_groups * 128, 6 * 128
    )
    xoh = np.concatenate([Xr, OHr], axis=1)
    return np.ascontiguousarray(xoh), recip, slot2bag


def _preprocess(x, scope, n_cores=N_CORES):
    n_sent = x.shape[0]
    n_bags = scope.shape[0] - 1
    scope = np.asarray(scope, dtype=np.int64)
    counts = np.diff(scope)
    assert counts.min() >= 1
    assert counts.max() <= ROWS16, "a small bag must fit a 256-row group"

    # bag-aligned row cuts near k * n_sent / n_cores
    bag_cuts = [0]
    for k in range(1, n_cores):
        t = (k * n_sent) // n_cores
        b = int(np.searchsorted(scope, t, side="right")) - 1
        bag_cuts.append(b)
    bag_cuts.append(n_bags)

    small = counts < SMALL_T
    per_core = []
    for c in range(n_cores):
        b0, b1 = bag_cuts[c], bag_cuts[c + 1]
        bag_ids = np.arange(b0, b1)
        sb = small[b0:b1]
        large_bags = bag_ids[~sb]
        small_bags = bag_ids[sb]
        _, ng8 = _pack(counts[large_bags], ROWS8, MAX_BAGS)
        _, nghl = _pack(counts[small_bags], ROWS8 // 2, MAX_BAGS)
        per_core.append((large_bags, small_bags, ng8, nghl))

    G8 = max(p[2] for p in per_core)
    GHL = max(p[3] for p in per_core)
    if (G8 + GHL) % 2:
        GHL += 1
    G = G8 + GHL

    cores = []
    for c in range(n_cores):
        large_bags, small_bags, _, _ = per_core[c]
        xoh8, recip8, s2b8 = _stream_arrays(
            x, scope, large_bags, counts, G8, hilo=False
        )
        xohl, recipl, s2bl = _stream_arrays(
            x, scope, small_bags, counts, GHL, hilo=True
        )
        xoh = np.concatenate([xoh8, xohl], axis=0)
        recip = np.concatenate([recip8, recipl], axis=0)
        slot2bag = np.concatenate([s2b8, s2bl], axis=0)
        cores.append(
            dict(
                xoh=xoh,
                recip=np.ascontiguousarray(recip.T),  # [128, G]
                slot2bag=slot2bag.reshape(-1),
            )
        )
    return cores, G


def _build_program(G):
    import concourse.bass as bass
    import concourse.mybir as mybir
    from concourse import tile

    dt = mybir.dt
    nc = bass.Bass()

    W8 = 6 * D + 6 * 128  # 4908 bytes per partition line (fp8)
    xoh_d = nc.declare_dram_parameter(
        "xoh", [G * 128, W8], dt.float8e4, isOutput=False
    )
    recip_d = nc.declare_dram_parameter("recip", [128, G], dt.float32, isOutput=False)
    ident_d = nc.declare_dram_parameter("ident", [128, 128], dt.float16, isOutput=False)
    wt_d = nc.declare_dram_parameter("wt", [128, 768], dt.float16, isOutput=False)
    bias_d = nc.declare_dram_parameter("bias", [C, 1], dt.float32, isOutput=False)
    out_d = nc.declare_dram_parameter("out", [C, G * 128], dt.float16, isOutput=True)

    DR = mybir.MatmulPerfMode.DoubleRow if USE_DR else None

    with tile.TileContext(nc) as tc:
        with (
            tc.tile_pool(name="const", bufs=1) as cpool,
            tc.tile_pool(name="x8in", bufs=8) as x8pool,
            tc.tile_pool(name="means", bufs=4) as mpool,
            tc.tile_pool(name="mgt", bufs=2) as tpool,
            tc.tile_pool(name="outs", bufs=2) as opool,
            tc.tile_pool(name="ps_sum", bufs=2, space="PSUM") as pspool,
            tc.tile_pool(name="ps_tr", bufs=1, space="PSUM") as ptpool,
            tc.tile_pool(name="ps_proj", bufs=1, space="PSUM") as pppool,
        ):
            ident_t = cpool.tile([128, 128], dt.float16)
            recip_t = cpool.tile([128, G], dt.float32)
            wt_t = cpool.tile([128, 768], dt.float16)
            bias_t = cpool.tile([C, 1], dt.float32)

            nc.gpsimd.dma_start(out=ident_t[:], in_=ident_d[:])
            nc.gpsimd.dma_start(out=recip_t[:], in_=recip_d[:])
            nc.gpsimd.dma_start(out=wt_t[:], in_=wt_d[:])
            nc.gpsimd.dma_start(out=bias_t[:], in_=bias_d[:])

            PREFETCH = 4

            def issue_dma(gg):
                x_t = x8pool.tile([128, W8], dt.float8e4, tag="x8", name="x_t")
                # alternate the two hardware DGE queues (SP / Activation)
                # so per-transfer queue overhead overlaps wire time
                dma_eng = nc.sync if gg % 2 == 0 else nc.scalar
                dma_eng.dma_start(
                    out=x_t[:], in_=xoh_d[gg * 128 : (gg + 1) * 128, :]
                )
                return x_t

            mgt = None
            x_q = {}  # group -> prefetched x tile
            means_q = {}  # group -> means tile, consumed two iterations later
            proj_q = {}  # pair -> mgt tile, projected one iteration later
            for g in range(G + 4):
                if g == 0:
                    for gg in range(min(PREFETCH + 1, G)):
                        x_q[gg] = issue_dma(gg)
                elif g + PREFETCH < G:
                    x_q[g + PREFETCH] = issue_dma(g + PREFETCH)
                if g < G:
                    ps = pspool.tile([128, 2 * 512], dt.float32, tag="ps")
                    x_t = x_q.pop(g)
                    for q in range(3):
                        oh = x_t[
                            :, 6 * D + q * 256 : 6 * D + (q + 1) * 256
                        ].rearrange("p (two m) -> p two m", two=2)
                        xr = x_t[:, q * 1380 : (q + 1) * 1380].rearrange(
                            "p (two d) -> p two d", two=2
                        )
                        for s, (c0, w) in enumerate(((0, 384), (384, 306))):
                            nc.tensor.matmul(
                                ps[:, s * 512 : s * 512 + w],
                                oh,
                                xr[:, :, c0 : c0 + w],
                                start=(q == 0),
                                stop=(q == 2),
                                perf_mode=DR,
                            )

                if g < G:
                    # means = psum * (1/count); scalar does bank 0, DVE
                    # does bank 1 (emitted later, after the copies)
                    means = mpool.tile([128, D], dt.float16, tag="m")
                    nc.scalar.activation(
                        means[:, 0:384],
                        ps[:, 0:384],
                        mybir.ActivationFunctionType.Copy,
                        scale=recip_t[:, g : g + 1],
                    )
                    means_q[g] = (means, ps)

                # transpose/copy for group g-2 (so the PE never waits on the
                # means activations), projection one iteration later still
                t = g - 2
                if t >= 0 and t < G:
                    means = means_q.pop(t)[0]
                    pt = ptpool.tile([128, 768], dt.float16, tag="pt")
                    for d in range(D_CHUNKS):
                        w = 128 if d < 5 else D_LAST
                        nc.tensor.transpose(
                            pt[0:w, d * 128 : d * 128 + 128],
                            means[:, d * 128 : d * 128 + w],
                            ident_t[:],
                        )
                    h = t % 2
                    if h == 0:
                        mgt = tpool.tile([128, 2 * 768], dt.float16, tag="mgt")
                    nc.vector.tensor_copy(
                        mgt[:, h * 768 : h * 768 + 640], pt[:, 0:640]
                    )
                    nc.vector.tensor_copy(
                        mgt[0:D_LAST, h * 768 + 640 : (h + 1) * 768],
                        pt[0:D_LAST, 640:768],
                    )
                    if h == 1:
                        proj_q[t // 2] = mgt

                if g < G:
                    # DVE half of this group's means, emitted after the
                    # copies so those aren't queued behind it on the DVE
                    means_g, ps_g = means_q[g]
                    nc.vector.tensor_scalar(
                        out=means_g[:, 384:D],
                        in0=ps_g[:, 512 : 512 + 306],
                        scalar1=recip_t[:, g : g + 1],
                        scalar2=None,
                        op0=mybir.AluOpType.mult,
                    )

                q2 = (g - 4) // 2
                if q2 in proj_q:
                    mgt_p = proj_q.pop(q2)
                    pp = pppool.tile([128, 256], dt.float32, tag="pp")
                    mgt3 = mgt_p.rearrange("p (two c) -> p two c", two=2)
                    for d in range(D_CHUNKS):
                        w = 128 if d < 5 else D_LAST
                        nc.tensor.matmul(
                            pp[:],
                            wt_t[0:w, d * 128 : (d + 1) * 128],
                            mgt3[0:w, :, d * 128 : d * 128 + 128],
                            start=(d == 0),
                            stop=(d == D_CHUNKS - 1),
                        )
                    out_sb = opool.tile([C, 256], dt.float16, tag="o")
                    nc.scalar.activation(
                        out_sb[:],
                        pp[0:C, :],
                        mybir.ActivationFunctionType.Identity,
                        bias=bias_t[:],
                    )
                    nc.sync.dma_start(
                        out=out_d[:, q2 * 256 : (q2 + 1) * 256], in_=out_sb[:]
                    )
    return nc


def prepare(x, scope, rel_weight, bias):
    """Build the SPMD program + per-core input maps. Returns a dict with
    everything needed to execute and assemble the output."""
    split_multi_waits, dedupe_ldweights = _apply_walrus_workarounds()

    x = np.asarray(x, dtype=np.float32)
    scope_np = np.asarray(scope)
    rel_weight = np.asarray(rel_weight, dtype=np.float32)
    bias = np.asarray(bias, dtype=np.float32)
    n_bags = scope_np.shape[0] - 1

    cores, G = _preprocess(x, scope_np)
    nc = _build_program(G)
    # NOTE: deduping back-to-back identical LDWEIGHTS is a pessimization:
    # walrus emits a self-loading matmul when an InstMatmult has no
    # preceding InstLdweights, serializing the weight load into every
    # matmul (~400ns vs ~262ns). Keep one LDW per matmul.
    split_multi_waits(nc)

    ident = np.eye(128, dtype=np.float16)
    wt = np.zeros((128, 768), dtype=np.float16)
    wpad = np.zeros((C, 768), dtype=np.float32)
    wpad[:, :D] = rel_weight
    for d in range(6):
        wt[:, d * 128 : d * 128 + C] = wpad[:, d * 128 : (d + 1) * 128].T
    bias_in = bias.reshape(C, 1).copy()

    in_maps = []
    for c in range(N_CORES):
        cd = cores[c]
        in_maps.append(
            {
                "xoh": cd["xoh"],
                "recip": cd["recip"],
                "ident": ident,
                "wt": wt,
                "bias": bias_in,
            }
        )

    def assemble(results):
        logits_t = np.zeros((C, n_bags), dtype=np.float32)
        for c in range(N_CORES):
            out = np.asarray(results[c]["out"], dtype=np.float32)
            s2b = cores[c]["slot2bag"]
            valid = s2b >= 0
            logits_t[:, s2b[valid]] = out[:, valid]
        return np.ascontiguousarray(logits_t.T)

    return dict(nc=nc, in_maps=in_maps, assemble=assemble, G=G)


def kernel(x, scope, rel_weight, bias):
    from concourse.bass_utils import run_bass_kernel_spmd

    p = prepare(x, scope, rel_weight, bias)
    res = run_bass_kernel_spmd(p["nc"], p["in_maps"], list(range(N_CORES)))
    return p["assemble"](res.results)


# revision 63
# speedup vs baseline: 1.1280x; 1.1280x over previous
"""Segment-mean + projection kernel for Trainium2 (8 NeuronCores, SPMD).

logits[b] = (mean of x rows in bag b) @ rel_weight.T + bias

Strategy: data-parallel over bags, two precision streams per core.

Large bags (count >= SMALL_T) go through an fp8-e4m3 path: rows are packed
into 768-row groups (3 pairs of 128-row tiles), the per-tile one-hot
matrices are precomputed on the host (exact 0/1 values in fp8) and DMA'd
alongside x, and the PE accumulates one-hot.T @ x with DoubleRow fp8
matmuls (two 128-row tiles contracted per pass at 0.5 cycles/column).
Small bags go through a classic fp16 path (256-row groups) because fp8
quantization error scales as 1/sqrt(count) and breaks the accuracy gate
for tiny bags.

Groups hold only whole bags (no bag straddles a group boundary), so there
is no fixup pass and no cross-group dependency. Per group the three
512-col PSUM bank regions are scaled by 1/count and copied to SBUF in one
activation, transposed chunk-wise by the PE into a single PSUM tile, and
copied once (768 cols) to the mgt staging buffer. Every two groups the
relation matrix is applied (6 accumulating matmuls) and bias added.
Host assembles the final [n_bags, 53] output from per-slot columns.
"""
import sys
import re

sys.path.insert(0, "/opt/trn_rl_repo")

import numpy as np
import ml_dtypes

N_CORES = 8
USE_DR = True  # DoubleRow fp8 matmuls
SMALL_T = 5  # bags with count < SMALL_T take the hi/lo path
PAIRS8 = 4  # DoubleRow pairs per full group (1024 rows)
PAIRS_HL = 3  # pairs per hi/lo group (384 rows at 2 tiles/row)
MAX_BAGS = 128  # output slots per group (PSUM partitions)
D = 690
SPLIT = 230  # 3 PSUM splits of 230 cols (each within one 2KB bank)
C = 53
D_CHUNKS = 6  # ceil(690 / 128); last chunk is 50 wide
D_LAST = D - 5 * 128  # 50

F8 = ml_dtypes.float8_e4m3


def _apply_walrus_workarounds():
    """This walrus build allows at most one semaphore wait per instruction
    on several opcodes (Drain, Matmult/LDW). Patch Tile's tail drain to use
    standalone wait_ge instructions, and provide a post-pass that hoists
    excess waits onto InstNoOp instructions."""
    from concourse import tile, mybir

    def _patched_drain_and_barrier(self, tick_clock, wait_clock):
        gc = tick_clock.global_clock
        ticks = [int(s) for s in re.findall(r"\d+", repr(gc))]
        allocated = self.sems.allocated()
        for proc, sem in sorted(allocated.items()):
            t = ticks[proc] if proc < len(ticks) else 0
            if t > 0:
                mult = 16 if "DMA" in sem.name else 1
                self.nc.sync.wait_ge(sem, t * mult)
        self.nc.sync.drain()
        self.nc.all_engine_barrier()
        popped = self.nc._tile_sem_poison_stack.pop()
        assert popped is self._sem_poison
        # One-shot outermost context: skip the semaphore RANGE_CLEAR and
        # the second barrier (only needed for NEFF re-execution / nested
        # contexts); just do the python-side bookkeeping.
        sem_nums = [s.num for s in allocated.values()]
        self.nc._state.prepend_free_semaphores(sem_nums)
        for poison_set in self.nc._tile_sem_poison_stack:
            poison_set.update(sem_nums)

    tile.TileContext._drain_and_barrier = _patched_drain_and_barrier

    def split_multi_waits(nc, max_waits=1):
        for f in nc.m.functions:
            for b in f.blocks:
                insts = list(b.instructions)
                new = []
                dirty = False
                for inst in insts:
                    si = inst.sync_info
                    if si is not None and len(si.on_wait) > max_waits:
                        waits = list(si.on_wait)
                        extra, keep = waits[:-max_waits], waits[-max_waits:]
                        for k, w in enumerate(extra):
                            nop = mybir.InstNoOp(
                                name=f"{inst.name}-hw{k}", ins=[], outs=[]
                            )
                            nop.engine = inst.engine
                            nop.sync_info = mybir.SyncInfo(
                                on_wait=[w], on_update=[]
                            )
                            new.append(nop)
                        inst.sync_info = mybir.SyncInfo(
                            on_wait=keep, on_update=list(si.on_update)
                        )
                        dirty = True
                    new.append(inst)
                if dirty:
                    b.instructions = new

    def dedupe_ldweights(nc):
        """Drop back-to-back InstLdweights that reload identical weights
        (the 2nd/3rd PSUM-split matmuls of a pair share the one-hot lhsT).
        The PE array retains weights across matmuls, LDWs neither wait nor
        tick tile semaphores (only the first of a run carries waits), so
        removal is safe and saves ~0.2us of PE issue per pair."""
        removed = 0
        for f in nc.m.functions:
            for b in f.blocks:
                out = []
                last_key = None
                for inst in b.instructions:
                    if isinstance(inst, mybir.InstLdweights):
                        ap = inst.ins[0]
                        key = (
                            str(getattr(ap, "memref", "")),
                            getattr(ap, "offset", None),
                            str(getattr(ap, "ap", "")),
                            str(getattr(ap, "dtype", "")),
                            str(getattr(inst, "perf_mode", None)),
                            str(getattr(inst, "tile_position", None)),
                        )
                        si = inst.sync_info
                        clean = si is None or (
                            not si.on_wait and not si.on_update
                        )
                        if key == last_key and clean:
                            removed += 1
                            continue
                        last_key = key
                    out.append(inst)
                b.instructions = out
        return removed

    return split_multi_waits, dedupe_ldweights


def _pack(cnts, max_rows, max_bags):
    """Greedy: consecutive bags into groups of <= max_rows rows and
    < max_bags bags. Returns per-bag group id and group count."""
    n = len(cnts)
    if n == 0:
        return np.zeros(0, np.int64), 0
    gids = np.zeros(n, np.int64)
    g = 0
    rows = 0
    nb = 0
    for i in range(n):
        c = int(cnts[i])
        if rows + c > max_rows or nb >= max_bags:
            g += 1
            rows = 0
            nb = 0
        gids[i] = g
        rows += c
        nb += 1
    return gids, g + 1


def _rows_of(scope, counts, stream_bags, gids, ng):
    """Per-row (group, rank-in-group, slot, global-row-id) for a stream."""
    nb = len(stream_bags)
    first = np.searchsorted(gids, np.arange(ng))
    slot_of_bag = np.arange(nb) - first[gids]
    cnts = counts[stream_bags]
    rows_per_g = np.bincount(gids, weights=cnts, minlength=ng).astype(np.int64)
    row_start_g = np.concatenate([[0], np.cumsum(rows_per_g)])[:-1]
    row_bag_rank = np.repeat(np.arange(nb), cnts)
    g_of_row = gids[row_bag_rank]
    n_rows = int(cnts.sum())
    row_rank = np.arange(n_rows) - row_start_g[g_of_row]
    bag_row0 = scope[stream_bags]
    within = np.arange(n_rows) - np.repeat(
        np.concatenate([[0], np.cumsum(cnts)])[:-1], cnts
    )
    grows = (bag_row0[row_bag_rank] + within).astype(np.int64)
    return g_of_row, row_rank, slot_of_bag[row_bag_rank], grows, slot_of_bag


def _stream_arrays(x, scope, stream_bags, counts, n_groups, hilo, n_pairs=3):
    """Build padded fp8 x + one-hot arrays for one stream of one core.

    Full stream (hilo=False): 768 rows/group, row slot r -> pair
    q = r // 256, tile j = (r // 128) % 2, partition p = r % 128.
    Hi/lo stream (hilo=True): 384 rows/group at double width, row
    r -> pair q = r // 128, partition p = r % 128; tile A holds
    e4m3(x), tile B holds e4m3(x - hi), the one-hot is duplicated in
    both halves so DoubleRow accumulates hi + lo = x at ~11-bit
    mantissa precision (for the small bags fp8 alone is too lossy).
    Returns (xoh [n_groups*128, 4908], recip [n_groups, 128],
    slot2bag [n_groups, 128]).
    """
    nb = len(stream_bags)
    rows_cap = n_pairs * (128 if hilo else 256)
    gids, ng = _pack(counts[stream_bags], rows_cap, MAX_BAGS)
    assert ng <= n_groups
    X = np.zeros((n_groups, n_pairs, 2, 128, D), dtype=F8)
    OH = np.zeros((n_groups, n_pairs, 2, 128, 128), dtype=F8)
    recip = np.ones((n_groups, 128), dtype=np.float32)
    slot2bag = np.full((n_groups, 128), -1, dtype=np.int64)
    if nb:
        g_of_row, row_rank, slot_of_row, grows, slot_of_bag = _rows_of(
            scope, counts, stream_bags, gids, ng
        )
        xs = x[grows]
        if hilo:
            hi = xs.astype(F8)
            lo = (xs - hi.astype(np.float32)).astype(F8)
            q, p = row_rank // 128, row_rank % 128
            X[g_of_row, q, 0, p] = hi
            X[g_of_row, q, 1, p] = lo
            OH[g_of_row, q, 0, p, slot_of_row] = 1.0
            OH[g_of_row, q, 1, p, slot_of_row] = 1.0
        else:
            q, j, p = row_rank // 256, (row_rank // 128) % 2, row_rank % 128
            X[g_of_row, q, j, p] = xs.astype(F8)
            OH[g_of_row, q, j, p, slot_of_row] = 1.0
        cnts = counts[stream_bags]
        recip[gids, slot_of_bag] = (1.0 / cnts).astype(np.float32)
        slot2bag[gids, slot_of_bag] = stream_bags
    Xr = np.ascontiguousarray(X.transpose(0, 3, 1, 2, 4)).reshape(
        n_groups * 128, n_pairs * 2 * D
    )
    OHr = np.ascontiguousarray(OH.transpose(0, 3, 1, 2, 4)).reshape(
        n_groups * 128, n_pairs * 2 * 128
    )
    xoh = np.concatenate([Xr, OHr], axis=1)
    return np.ascontiguousarray(xoh), recip, slot2bag


def _preprocess(x, scope, n_cores=N_CORES):
    n_sent = x.shape[0]
    n_bags = scope.shape[0] - 1
    scope = np.asarray(scope, dtype=np.int64)
    counts = np.diff(scope)
    assert counts.min() >= 1
    assert counts.max() <= PAIRS8 * 256, "a bag must fit one group"

    # bag-aligned row cuts near k * n_sent / n_cores
    bag_cuts = [0]
    for k in range(1, n_cores):
        t = (k * n_sent) // n_cores
        b = int(np.searchsorted(scope, t, side="right")) - 1
        bag_cuts.append(b)
    bag_cuts.append(n_bags)

    small = counts < SMALL_T
    per_core = []
    for c in range(n_cores):
        b0, b1 = bag_cuts[c], bag_cuts[c + 1]
        bag_ids = np.arange(b0, b1)
        sb = small[b0:b1]
        large_bags = bag_ids[~sb]
        small_bags = bag_ids[sb]
        # zipper order (largest, smallest, 2nd-largest, ...) so greedy
        # packing fills groups to both caps (128 bags AND the row cap)
        # instead of hitting one cap with the other half-empty
        def _zipper(bags):
            srt = bags[np.argsort(-counts[bags], kind="stable")]
            z = np.empty_like(srt)
            h = (len(srt) + 1) // 2
            z[0::2] = srt[:h]
            z[1::2] = srt[len(srt) - 1 : h - 1 : -1]
            return z

        small_bags = _zipper(small_bags)
        large_bags = _zipper(large_bags)
        _, ng8 = _pack(counts[large_bags], PAIRS8 * 256, MAX_BAGS)
        _, nghl = _pack(counts[small_bags], PAIRS_HL * 128, MAX_BAGS)
        per_core.append((large_bags, small_bags, ng8, nghl))

    G8 = max(p[2] for p in per_core)
    GHL = max(p[3] for p in per_core)
    if (G8 + GHL) % 2:
        GHL += 1
    G = G8 + GHL

    cores = []
    for c in range(n_cores):
        large_bags, small_bags, _, _ = per_core[c]
        xoh8, recip8, s2b8 = _stream_arrays(
            x, scope, large_bags, counts, G8, hilo=False, n_pairs=PAIRS8
        )
        xohl, recipl, s2bl = _stream_arrays(
            x, scope, small_bags, counts, GHL, hilo=True, n_pairs=PAIRS_HL
        )
        recip = np.concatenate([recip8, recipl], axis=0)
        slot2bag = np.concatenate([s2b8, s2bl], axis=0)
        cores.append(
            dict(
                xoh8=xoh8,
                xohl=xohl,
                recip=np.ascontiguousarray(recip.T),  # [128, G]
                slot2bag=slot2bag.reshape(-1),
            )
        )
    return cores, G8, GHL


def _build_program(G8, GHL):
    import concourse.bass as bass
    import concourse.mybir as mybir
    from concourse import tile

    dt = mybir.dt
    G = G8 + GHL
    nc = bass.Bass()

    W8 = PAIRS8 * 2 * D + PAIRS8 * 256  # full-group partition line (fp8)
    WHL = PAIRS_HL * 2 * D + PAIRS_HL * 256  # hi/lo-group line
    xoh8_d = nc.declare_dram_parameter(
        "xoh8", [G8 * 128, W8], dt.float8e4, isOutput=False
    )
    xohl_d = nc.declare_dram_parameter(
        "xohl", [GHL * 128, WHL], dt.float8e4, isOutput=False
    )
    recip_d = nc.declare_dram_parameter("recip", [128, G], dt.float32, isOutput=False)
    ident_d = nc.declare_dram_parameter("ident", [128, 128], dt.float16, isOutput=False)
    wt_d = nc.declare_dram_parameter("wt", [128, 768], dt.float16, isOutput=False)
    bias_d = nc.declare_dram_parameter("bias", [C, 1], dt.float32, isOutput=False)
    out_d = nc.declare_dram_parameter("out", [C, G * 128], dt.float16, isOutput=True)

    DR = mybir.MatmulPerfMode.DoubleRow if USE_DR else None

    with tile.TileContext(nc) as tc:
        with (
            tc.tile_pool(name="const", bufs=1) as cpool,
            tc.tile_pool(name="x8in", bufs=8) as x8pool,
            tc.tile_pool(name="means", bufs=4) as mpool,
            tc.tile_pool(name="mgt", bufs=2) as tpool,
            tc.tile_pool(name="outs", bufs=2) as opool,
            tc.tile_pool(name="ps_sum", bufs=2, space="PSUM") as pspool,
            tc.tile_pool(name="ps_tr", bufs=1, space="PSUM") as ptpool,
            tc.tile_pool(name="ps_proj", bufs=1, space="PSUM") as pppool,
        ):
            ident_t = cpool.tile([128, 128], dt.float16)
            recip_t = cpool.tile([128, G], dt.float32)
            wt_t = cpool.tile([128, 768], dt.float16)
            bias_t = cpool.tile([C, 1], dt.float32)

            nc.gpsimd.dma_start(out=ident_t[:], in_=ident_d[:])
            nc.gpsimd.dma_start(out=recip_t[:], in_=recip_d[:])
            nc.gpsimd.dma_start(out=wt_t[:], in_=wt_d[:])
            nc.gpsimd.dma_start(out=bias_t[:], in_=bias_d[:])

            PREFETCH = 4

            def issue_dma(gg):
                # alternate the two hardware DGE queues (SP / Activation)
                # so per-transfer queue overhead overlaps wire time
                dma_eng = nc.sync if gg % 2 == 0 else nc.scalar
                if gg < G8:
                    x_t = x8pool.tile(
                        [128, W8], dt.float8e4, tag="x8", name="x_t"
                    )
                    dma_eng.dma_start(
                        out=x_t[:], in_=xoh8_d[gg * 128 : (gg + 1) * 128, :]
                    )
                else:
                    hh = gg - G8
                    x_t = x8pool.tile(
                        [128, WHL], dt.float8e4, tag="xhl", name="x_t"
                    )
                    dma_eng.dma_start(
                        out=x_t[:], in_=xohl_d[hh * 128 : (hh + 1) * 128, :]
                    )
                return x_t

            mgt = None
            x_q = {}  # group -> prefetched x tile
            means_q = {}  # group -> means tile, consumed two iterations later
            proj_q = {}  # pair -> mgt tile, projected one iteration later
            for g in range(G + 4):
                if g == 0:
                    for gg in range(min(PREFETCH + 1, G)):
                        x_q[gg] = issue_dma(gg)
                elif g + PREFETCH < G:
                    x_q[g + PREFETCH] = issue_dma(g + PREFETCH)
                if g < G:
                    ps = pspool.tile([128, 2 * 512], dt.float32, tag="ps")
                    x_t = x_q.pop(g)
                    npair = PAIRS8 if g < G8 else PAIRS_HL
                    oh_base = npair * 2 * D
                    for q in range(npair):
                        oh = x_t[
                            :, oh_base + q * 256 : oh_base + (q + 1) * 256
                        ].rearrange("p (two m) -> p two m", two=2)
                        xr = x_t[:, q * 1380 : (q + 1) * 1380].rearrange(
                            "p (two d) -> p two d", two=2
                        )
                        for s, (c0, w) in enumerate(((0, 384), (384, 306))):
                            nc.tensor.matmul(
                                ps[:, s * 512 : s * 512 + w],
                                oh,
                                xr[:, :, c0 : c0 + w],
                                start=(q == 0),
                                stop=(q == npair - 1),
                                perf_mode=DR,
                            )

                if g < G:
                    # means = psum * (1/count); scalar does bank 0, DVE
                    # does bank 1 (emitted later, after the copies)
                    means = mpool.tile([128, D], dt.float16, tag="m")
                    nc.scalar.activation(
                        means[:, 0:384],
                        ps[:, 0:384],
                        mybir.ActivationFunctionType.Copy,
                        scale=recip_t[:, g : g + 1],
                    )
                    means_q[g] = (means, ps)

                # transpose/copy for group g-2 (so the PE never waits on the
                # means activations), projection one iteration later still
                t = g - 2
                if t >= 0 and t < G:
                    means = means_q.pop(t)[0]
                    pt = ptpool.tile([128, 768], dt.float16, tag="pt")
                    for d in range(D_CHUNKS):
                        w = 128 if d < 5 else D_LAST
                        nc.tensor.transpose(
                            pt[0:w, d * 128 : d * 128 + 128],
                            means[:, d * 128 : d * 128 + w],
                            ident_t[:],
                        )
                    h = t % 2
                    if h == 0:
                        mgt = tpool.tile([128, 2 * 768], dt.float16, tag="mgt")
                    nc.vector.tensor_copy(
                        mgt[:, h * 768 : h * 768 + 640], pt[:, 0:640]
                    )
                    nc.vector.tensor_copy(
                        mgt[0:D_LAST, h * 768 + 640 : (h + 1) * 768],
                        pt[0:D_LAST, 640:768],
                    )
                    if h == 1:
                        proj_q[t // 2] = mgt

                if g < G:
                    # DVE half of this group's means, emitted after the
                    # copies so those aren't queued behind it on the DVE
                    means_g, ps_g = means_q[g]
                    nc.vector.tensor_scalar(
                        out=means_g[:, 384:D],
                        in0=ps_g[:, 512 : 512 + 306],
                        scalar1=recip_t[:, g : g + 1],
                        scalar2=None,
                        op0=mybir.AluOpType.mult,
                    )

                q2 = (g - 4) // 2
                if q2 in proj_q:
                    mgt_p = proj_q.pop(q2)
                    pp = pppool.tile([128, 256], dt.float32, tag="pp")
                    mgt3 = mgt_p.rearrange("p (two c) -> p two c", two=2)
                    for d in range(D_CHUNKS):
                        w = 128 if d < 5 else D_LAST
                        nc.tensor.matmul(
                            pp[:],
                            wt_t[0:w, d * 128 : (d + 1) * 128],
                            mgt3[0:w, :, d * 128 : d * 128 + 128],
                            start=(d == 0),
                            stop=(d == D_CHUNKS - 1),
                        )
                    out_sb = opool.tile([C, 256], dt.float16, tag="o")
                    nc.scalar.activation(
                        out_sb[:],
                        pp[0:C, :],
                        mybir.ActivationFunctionType.Identity,
                        bias=bias_t[:],
                    )
                    nc.gpsimd.dma_start(
                        out=out_d[:, q2 * 256 : (q2 + 1) * 256], in_=out_sb[:]
                    )
    return nc


def prepare(x, scope, rel_weight, bias):
    """Build the SPMD program + per-core input maps. Returns a dict with
    everything needed to execute and assemble the output."""
    split_multi_waits, dedupe_ldweights = _apply_walrus_workarounds()

    x = np.asarray(x, dtype=np.float32)
    scope_np = np.asarray(scope)
    rel_weight = np.asarray(rel_weight, dtype=np.float32)
    bias = np.asarray(bias, dtype=np.float32)
    n_bags = scope_np.shape[0] - 1

    cores, G8, GHL = _preprocess(x, scope_np)
    G = G8 + GHL
    nc = _build_program(G8, GHL)
    # NOTE: deduping back-to-back identical LDWEIGHTS is a pessimization:
    # walrus emits a self-loading matmul when an InstMatmult has no
    # preceding InstLdweights, serializing the weight load into every
    # matmul (~400ns vs ~262ns). Keep one LDW per matmul.
    split_multi_waits(nc)

    ident = np.eye(128, dtype=np.float16)
    wt = np.zeros((128, 768), dtype=np.float16)
    wpad = np.zeros((C, 768), dtype=np.float32)
    wpad[:, :D] = rel_weight
    for d in range(6):
        wt[:, d * 128 : d * 128 + C] = wpad[:, d * 128 : (d + 1) * 128].T
    bias_in = bias.reshape(C, 1).copy()

    in_maps = []
    for c in range(N_CORES):
        cd = cores[c]
        in_maps.append(
            {
                "xoh8": cd["xoh8"],
                "xohl": cd["xohl"],
                "recip": cd["recip"],
                "ident": ident,
                "wt": wt,
                "bias": bias_in,
            }
        )

    def assemble(results):
        logits_t = np.zeros((C, n_bags), dtype=np.float32)
        for c in range(N_CORES):
            out = np.asarray(results[c]["out"], dtype=np.float32)
            s2b = cores[c]["slot2bag"]
            valid = s2b >= 0
            logits_t[:, s2b[valid]] = out[:, valid]
        return np.ascontiguousarray(logits_t.T)

    return dict(nc=nc, in_maps=in_maps, assemble=assemble, G=G)


def kernel(x, scope, rel_weight, bias):
    from concourse.bass_utils import run_bass_kernel_spmd

    p = prepare(x, scope, rel_weight, bias)
    res = run_bass_kernel_spmd(p["nc"], p["in_maps"], list(range(N_CORES)))
    return p["assemble"](res.results)
